# revision 44
# baseline (speedup 1.0000x reference)
"""MoE (64 experts, top-24) on 8 Trainium2 NeuronCores — sparse dispatch.

Data-parallel shard of the 8192-token batch (1024 tokens/core). The
reference computes a dense all-expert MLP but only the top-24 experts per
token reach the output, so each core computes only the selected
(token, expert) pairs (~37.5% of the dense matmul work):

  - gate logits in exact fp32 + top-24 masked-softmax routing via DVE
    max8/match_replace (identical to the dense baseline),
  - on-device dispatch-map build: per-expert exclusive-cumsum positions
    via PE triangular matmuls, one dma_scatter_add (CCE add) into a DRAM
    scratch map (slot -> (token_id+1, gating)) over a (-1, 0) init, read
    back with affine APs as wrapped-16 int16 gather index lists and
    per-slot gating in partition-major layout,
  - per expert e: dma_gather(transpose=True) pulls the c_e selected token
    rows of x (bf16) from HBM directly into the transposed
    [d%128, d//128, slot] layout; L1/L2 run in bf16 at full PE rate over
    CAP=512 static slots (pads have gating 0, idx -1; num_idxs_reg=c_e
    keeps pad DMA off the wire),
  - L2 emits token-ROW-major h2 [slot, o] (stationary = s1 slot block),
    gating applied as per-partition scale at PSUM evacuation,
  - combine: dma_scatter_add with SBUF parity-split destination (CCE add)
    accumulates h2 rows into even/odd-token-tile SBUF accumulators; the
    b2 term (sum_k w_k b2[e_k]) is pre-seeded by a small PE matmul. Four
    rotating accumulator sets keep CCE WAW chains off the critical path.
"""

import sys
import types

import numpy as np

try:
    import ml_dtypes
    _BF16 = ml_dtypes.bfloat16
except ImportError:  # pragma: no cover
    _BF16 = None

import concourse.bass as bass
import concourse.tile as tile
import concourse.mybir as mybir
from concourse import bacc, bass_utils
from concourse.bass import AP

# bass_utils imports antenv.axon_hooks when BASS_TRACE=1; some images lack it.
try:
    import antenv.axon_hooks  # noqa: F401
except ImportError:
    try:
        import contextlib
        import ctypes

        def _make_hook():
            try:
                lib = ctypes.CDLL("/opt/axon/libaxon_pjrt.so")
            except OSError:
                return None
            if not hasattr(lib, "axon_start_nrt_profile"):
                return None
            lib.axon_start_nrt_profile.argtypes = [
                ctypes.POINTER(ctypes.c_int64), ctypes.c_size_t]
            lib.axon_start_nrt_profile.restype = ctypes.c_int64
            lib.axon_stop_nrt_profile.argtypes = [ctypes.c_char_p]
            lib.axon_stop_nrt_profile.restype = ctypes.c_int64

            @contextlib.contextmanager
            def _hook(output_dir, device_ids):
                import jax
                jax.devices()
                if device_ids:
                    ids = (ctypes.c_int64 * len(device_ids))(*device_ids)
                    rc = lib.axon_start_nrt_profile(ids, len(device_ids))
                else:
                    rc = lib.axon_start_nrt_profile(None, 0)
                if rc != 0:
                    raise RuntimeError(f"axon_start_nrt_profile rc={rc}")
                try:
                    yield
                finally:
                    lib.axon_stop_nrt_profile(str(output_dir).encode())

            return _hook

        _mod = types.ModuleType("antenv.axon_hooks")
        _mod.get_axon_ntff_profile_hook = _make_hook
        _mod.set_axon_ntff_profile_hook = lambda h: None
        sys.modules["antenv.axon_hooks"] = _mod
    except Exception:
        pass

F32 = mybir.dt.float32
BF16 = mybir.dt.bfloat16
I16 = mybir.dt.int16
I32 = mybir.dt.int32
AF = mybir.ActivationFunctionType
ALU = mybir.AluOpType
AX = mybir.AxisListType

NCORES = 8
B = 8192
D = 1024          # input dim
H = 256           # hidden dim
O = 256           # output dim
NE = 64           # experts
TOPK = 24
BS = B // NCORES  # tokens per core (1024)
NBT = BS // 128   # b-tiles per core (8)
NG = BS // 512    # 512-token groups per core (2)
KC = D // 128     # contraction chunks for layer 1 (8)
HC = H // 128     # contraction chunks for layer 2 (2)
CAP = 512         # per-expert slot capacity (gather stride; %128 required)
CAPC = 448        # compute width: max actual count ~430 (host-checked)
NT = CAP // 128   # slot tiles per expert (4)
L2W = [128, 128, 128, CAPC - 384]   # L2 stationary tile widths
EPG = 4           # experts per gather/scatter group
NGRP = NE // EPG  # 16 groups
NACC = 2          # rotating accumulator sets

_CACHE = {}


def _build(ndev=NCORES):
    nc = bacc.Bacc("TRN2", target_bir_lowering=False, debug=False,
                   num_devices=ndev)

    xt_d = nc.dram_tensor("xt", (D, BS), F32, kind="ExternalInput").ap()
    xb_d = nc.dram_tensor("xb", (BS, D), BF16, kind="ExternalInput").ap()
    gw_d = nc.dram_tensor("gw", (D, NE), F32, kind="ExternalInput").ap()
    gb_d = nc.dram_tensor("gb", (NE, 1), F32, kind="ExternalInput").ap()
    w1_d = nc.dram_tensor("w1", (NE, D, H), BF16, kind="ExternalInput").ap()
    b1_d = nc.dram_tensor("b1p", (128, HC * NE), F32, kind="ExternalInput").ap()
    w2_d = nc.dram_tensor("w2", (NE, H, O), BF16, kind="ExternalInput").ap()
    b2_d = nc.dram_tensor("b2", (NE, O), F32, kind="ExternalInput").ap()
    # host-provided constants
    id_d = nc.dram_tensor("idn", (128, 128), F32, kind="ExternalInput").ap()
    u_d = nc.dram_tensor("u129", (128, 130), BF16, kind="ExternalInput").ap()
    it_d = nc.dram_tensor("iot16", (NE, BS), I16, kind="ExternalInput").ap()
    e_d = nc.dram_tensor("emat", (128, 8, 128), F32, kind="ExternalInput").ap()
    out_d = nc.dram_tensor("out", (BS, O), F32, kind="ExternalOutput").ap()

    with tile.TileContext(nc) as tc:
        with tc.tile_pool(name="res", bufs=1) as res, \
             tc.tile_pool(name="w1p", bufs=3) as w1p, \
             tc.tile_pool(name="w2p", bufs=3) as w2p, \
             tc.tile_pool(name="s1p", bufs=2) as s1p, \
             tc.tile_pool(name="h2p", bufs=2) as h2p, \
             tc.tile_pool(name="rt", bufs=2) as rt, \
             tc.tile_pool(name="phA", bufs=4, space="PSUM") as phA, \
             tc.tile_pool(name="phB", bufs=2, space="PSUM") as phB:
            xtp_cm = tc.tile_pool(name="xtp", bufs=1)
            xtp = xtp_cm.__enter__()

            # ---------------- resident loads ----------------
            gw_sb = res.tile([128, KC, NE], F32)
            nc.sync.dma_start(gw_sb[:], gw_d.rearrange("(c p) n -> p c n", p=128))
            gb_sb = res.tile([NE, 1], F32)
            nc.sync.dma_start(gb_sb[:], gb_d[:])

            xt_f = xtp.tile([128, KC, BS], F32)       # gate moving operand
            for ic in range(KC):
                nc.sync.dma_start(xt_f[:, ic, :],
                                  xt_d[ic * 128:(ic + 1) * 128, :])

            def load_expert(e):
                w1_t = w1p.tile([128, KC, H], BF16, tag="w1", name=f"w1_{e}")
                nc.sync.dma_start(
                    w1_t[:], w1_d[e].rearrange("(c p) h -> p c h", p=128))
                w2_t = w2p.tile([128, HC, O], BF16, tag="w2", name=f"w2_{e}")
                nc.sync.dma_start(
                    w2_t[:], w2_d[e].rearrange("(c p) o -> p c o", p=128))
                return w1_t, w2_t

            preload = {0: load_expert(0), 1: load_expert(1)}

            b1_sb = res.tile([128, HC * NE], F32)
            nc.sync.dma_start(b1_sb[:], b1_d[:])
            b2_sb = res.tile([NE, O], F32)
            nc.sync.dma_start(b2_sb[:], b2_d[:])
            ident = res.tile([128, 128], F32)
            nc.sync.dma_start(ident[:], id_d[:])
            u_sb = res.tile([128, 130], BF16)
            nc.sync.dma_start(u_sb[:], u_d[:])
            iota16 = res.tile([NE, BS], I16)
            nc.sync.dma_start(iota16[:], it_d[:])
            em_sb = res.tile([128, 8, 128], F32)
            nc.sync.dma_start(em_sb[:], e_d[:])

            # NOTE: gather pad columns (beyond round16(c_e)) keep stale SBUF
            # bits; any resulting NaNs stay confined to pad slots whose h2
            # rows are never scattered (num_idxs_reg = c_e).

            # accumulators: NACC sets x (even, odd); group 4 = trash
            acc = []
            for s in range(NACC):
                ae = res.tile([128, NBT // 2 + 1, O], F32, name=f"acc_e{s}")
                ao = res.tile([128, NBT // 2 + 1, O], F32, name=f"acc_o{s}")
                if s > 0:
                    nc.vector.memset(ae[:, 0:NBT // 2, :], 0.0)
                    nc.vector.memset(ao[:, 0:NBT // 2, :], 0.0)
                acc.append((ae, ao))

            g_sb = res.tile([128, NBT, NE], F32)       # gate logits
            wroute = res.tile([128, NBT, NE], F32)     # routing weights
            wrouteT = res.tile([NE, NBT, 128], F32)
            m_tp = res.tile([128, NBT, NE], BF16)      # mask, token-major

            # ---------------- gate logits (exact fp32) ----------------
            gT_sb = res.tile([64, NG, 512], F32)
            for g in range(NG):
                pgt = phA.tile([128, 512], F32, tag="phA", name=f"pgt_{g}")
                for ic in range(KC):
                    nc.tensor.matmul(
                        pgt[0:NE, :],
                        gw_sb[:, ic, :],
                        xt_f[:, ic, g * 512:(g + 1) * 512],
                        start=(ic == 0), stop=(ic == KC - 1))
                nc.scalar.activation(gT_sb[:, g, :], pgt[0:NE, :],
                                     AF.Identity, bias=gb_sb[:], scale=1.0)
                for btl in range(4):
                    bt = g * 4 + btl
                    ptg = phA.tile([128, 512], F32, tag="phA",
                                   name=f"ptg_{bt}")
                    nc.tensor.transpose(
                        ptg[:, 0:NE],
                        gT_sb[:, g, btl * 128:(btl + 1) * 128],
                        ident[0:NE, 0:NE])
                    nc.scalar.copy(g_sb[:, bt, :], ptg[:, 0:NE])

            xtp_cm.__exit__(None, None, None)
            xgp_cm = tc.tile_pool(name="xgp", bufs=4)
            xgp = xgp_cm.__enter__()

            # ---------------- top-24 masked softmax ----------------
            for bt in range(NBT):
                g = g_sb[:, bt, :]
                m8 = rt.tile([128, 3, 8], F32, tag="m8")
                gwk = rt.tile([128, 3, NE], F32, tag="gwk")
                nc.vector.max(m8[:, 0, :], g)
                nc.vector.match_replace(gwk[:, 0, :], m8[:, 0, :], g, -1e30)
                nc.vector.max(m8[:, 1, :], gwk[:, 0, :])
                nc.vector.match_replace(gwk[:, 1, :], m8[:, 1, :], gwk[:, 0, :], -1e30)
                nc.vector.max(m8[:, 2, :], gwk[:, 1, :])
                nc.vector.match_replace(gwk[:, 2, :], m8[:, 2, :], gwk[:, 1, :], -1e30)
                maskt = rt.tile([128, NE], F32, tag="maskt")
                nc.vector.tensor_scalar(maskt[:], gwk[:, 2, :], -1e29, None,
                                        op0=ALU.is_lt)
                nc.vector.tensor_copy(m_tp[:, bt, :], maskt[:])  # bf16 cast
                negm1 = rt.tile([128, 1], F32, tag="negm1")
                nc.vector.tensor_scalar_mul(negm1[:], m8[:, 0, 0:1], -1.0)
                e_sb = rt.tile([128, NE], F32, tag="e_sb")
                nc.scalar.activation(e_sb[:], g, AF.Exp, bias=negm1[:], scale=1.0)
                em = rt.tile([128, NE], F32, tag="em")
                nc.vector.tensor_mul(em[:], e_sb[:], maskt[:])
                ssum = rt.tile([128, 1], F32, tag="ssum")
                nc.vector.reduce_sum(ssum[:], em[:], axis=AX.X)
                rsum = rt.tile([128, 1], F32, tag="rsum")
                nc.vector.reciprocal(rsum[:], ssum[:])
                nc.vector.tensor_scalar_mul(wroute[:, bt, :], em[:], rsum[:])

            # wrouteT [e, bt, t]
            for bt in range(NBT):
                ptr_ = phA.tile([128, 512], F32, tag="phA", name=f"ptr_{bt}")
                nc.tensor.transpose(ptr_[0:64, 0:128], wroute[:, bt, :],
                                    ident[:])
                nc.scalar.copy(wrouteT[:, bt, :], ptr_[0:64, 0:128])

            # ---------------- dispatch-map build ----------------
            # per-expert exclusive cumsum over tokens (PE triangular matmul
            # per 128-token block; col 128 of u_sb = ones -> block counts)
            posblk = res.tile([NE, NBT, 130], F32)
            for bt in range(NBT):
                pp = phA.tile([128, 512], F32, tag="phA", name=f"pos_{bt}")
                nc.tensor.matmul(pp[0:NE, 0:130], m_tp[:, bt, :],
                                 u_sb[:, 0:130], start=True, stop=True)
                nc.scalar.copy(posblk[:, bt, :], pp[0:NE, 0:130])

            off = res.tile([NE, NBT], F32)       # exclusive block offsets
            nc.vector.memset(off[:, 0:1], 0.0)
            for bt in range(1, NBT):
                nc.vector.tensor_add(off[:, bt:bt + 1], off[:, bt - 1:bt],
                                     posblk[:, bt - 1, 128:129])

            pos_em = res.tile([NE, NBT, 128], F32)
            nc.vector.tensor_tensor(
                pos_em[:], posblk[:, :, 0:128],
                off[:].unsqueeze(2).to_broadcast([NE, NBT, 128]),
                op=ALU.add)
            m_em = res.tile([NE, NBT, 128], F32)
            nc.vector.tensor_scalar(m_em[:], wrouteT[:], 0.0, None,
                                    op0=ALU.is_gt)
            # pos_sel = m ? pos : -1  =  pos*m + m - 1
            nc.vector.tensor_mul(pos_em[:], pos_em[:], m_em[:])
            nc.vector.tensor_add(pos_em[:], pos_em[:], m_em[:])
            nc.vector.tensor_scalar(pos_em[:], pos_em[:], -1.0, None,
                                    op0=ALU.add)
            pos_i = res.tile([NE, BS], I16)
            nc.vector.tensor_copy(pos_i[:],
                                  pos_em[:].rearrange("e b t -> e (b t)"))

            # per-expert slot maps, entirely on-chip (gpsimd local_scatter)
            sel_lin = res.tile([NE, CAP], I16)    # token id + 1, 0 pads
            nc.gpsimd.local_scatter(sel_lin[:], iota16[:], pos_i[:],
                                    channels=NE, num_elems=CAP, num_idxs=BS)
            wbf = res.tile([NE, BS], BF16)
            nc.vector.tensor_copy(wbf[:],
                                  wrouteT[:].rearrange("e b t -> e (b t)"))
            g_lin = res.tile([NE, CAP], BF16)     # gating per slot, 0 pads
            nc.gpsimd.local_scatter(g_lin[:], wbf[:], pos_i[:],
                                    channels=NE, num_elems=CAP, num_idxs=BS)

            # g_pm [p, e, tile]: gating partition-major via 4 PE transposes
            g_f = res.tile([NE, CAP], F32)
            nc.vector.tensor_copy(g_f[:], g_lin[:])
            g_pm = res.tile([128, NE, NT], F32)
            for b in range(NT):
                pt = phA.tile([128, 512], F32, tag="phA", name=f"gpt_{b}")
                nc.tensor.transpose(pt[:, 0:NE],
                                    g_f[:, b * 128:(b + 1) * 128],
                                    ident[0:NE, 0:NE])
                nc.scalar.copy(g_pm[:, :, b], pt[:, 0:NE])

            # sel16 [p, e, j]: wrapped-16 gather indices.
            # sel_f = sel_lin - 1 (f32: token id, -1 pads), transpose blocks
            # to slot-major, then E-matmuls pick rows 16q+(p%16) (replicated).
            sel_f = res.tile([NE, CAP], F32)
            nc.vector.tensor_copy(sel_f[:], sel_lin[:])
            nc.vector.tensor_scalar(sel_f[:], sel_f[:], -1.0, None,
                                    op0=ALU.add)
            slT = res.tile([128, NT, NE], F32)
            for b in range(NT):
                pt = phA.tile([128, 512], F32, tag="phA", name=f"slt_{b}")
                nc.tensor.transpose(pt[:, 0:NE],
                                    sel_f[:, b * 128:(b + 1) * 128],
                                    ident[0:NE, 0:NE])
                nc.scalar.copy(slT[:, b, :], pt[:, 0:NE])
            sel16 = res.tile([128, NE, CAP // 16], I16)
            sel16s = res.tile([128, NE, CAP // 16], I16)
            for b in range(NT):
                pw = phA.tile([128, 512], F32, tag="phA", name=f"pw_{b}")
                for q in range(8):
                    nc.tensor.matmul(
                        pw[:, q * NE:(q + 1) * NE],
                        em_sb[:, q, :],
                        slT[:, b, :],
                        start=True, stop=True, skip_group_check=True)
                # psum [p, q, e] -> sel16 [p, e, jj=b*8+q] (strided DVE cast)
                nc.vector.tensor_copy(
                    sel16[:, :, b * 8:(b + 1) * 8],
                    pw[:].rearrange("p (q e) -> p q e", q=8)
                    .transpose([0, 2, 1]))
                # scatter variant: pads (-1) -> trash token 1024
                swr = rt.tile([128, 512], F32, tag="swr", name=f"swr_{b}")
                nc.vector.tensor_scalar(swr[:], pw[:], 0.0, None,
                                        op0=ALU.is_lt)
                nc.vector.tensor_scalar(swr[:], swr[:], 1025.0, None,
                                        op0=ALU.mult)
                nc.vector.tensor_add(swr[:], swr[:], pw[:])
                nc.vector.tensor_copy(
                    sel16s[:, :, b * 8:(b + 1) * 8],
                    swr[:].rearrange("p (q e) -> p q e", q=8)
                    .transpose([0, 2, 1]))

            # b2 term into accumulator set 0
            for half in range(2):
                pb2 = phB.tile([128, NT, O], F32, tag="phB",
                               name=f"pb2_{half}")
                for btl in range(4):
                    bt = half * 4 + btl
                    nc.tensor.matmul(pb2[:, btl, :], wrouteT[:, bt, :],
                                     b2_sb[:], start=True, stop=True,
                                     skip_group_check=True)
                    nc.scalar.copy(acc[0][bt % 2][:, bt // 2, :],
                                   pb2[:, btl, :])

            # gather index lists with pads -> row 0 (all-valid, static count)
            sel16g = res.tile([128, NE, CAP // 16], I16)
            nc.vector.tensor_scalar_max(sel16g[:], sel16[:], 0)

            # ---------------- expert loop (4-expert groups) ----------------

            def emit_gather(g):
                xgs = []
                for el in range(EPG):
                    e = g * EPG + el
                    xg = xgp.tile([128, KC, CAP], BF16, tag="xg",
                                  name=f"xg_{e}")
                    nc.gpsimd.dma_gather(
                        xg[:], xb_d[:, :], sel16g[:, e, :],
                        num_idxs=CAP, num_idxs_reg=CAP, elem_size=D,
                        transpose=True)
                    xgs.append(xg)
                return xgs

            gathered = {0: emit_gather(0), 1: emit_gather(1)}

            for g in range(NGRP):
                if g + 2 < NGRP:
                    gathered[g + 2] = emit_gather(g + 2)
                xg = gathered.pop(g)
                h2g = h2p.tile([128, EPG, NT, O], F32, tag="h2g",
                               name=f"h2g_{g}")
                # slots >= CAPC of the last L2 tile: zero (never computed)
                nc.vector.memset(h2g[L2W[NT - 1]:128, :, NT - 1, :], 0.0)
                for el in range(EPG):
                    e = g * EPG + el
                    w1_t, w2_t = preload[e] if e in preload else load_expert(e)
                    # L1: h1T [h, slot], compute width CAPC
                    ph1 = [phA.tile([128, 512], F32, tag="phA",
                                    name=f"ph1_{e}_{hc}") for hc in range(HC)]
                    for hc in range(HC):
                        for ic in range(KC):
                            nc.tensor.matmul(
                                ph1[hc][:, 0:CAPC],
                                w1_t[:, ic, hc * 128:(hc + 1) * 128],
                                xg[el][:, ic, 0:CAPC],
                                start=(ic == 0), stop=(ic == KC - 1))
                    s1 = s1p.tile([128, HC, CAPC], BF16, tag="s1",
                                  name=f"s1_{e}")
                    for hc in range(HC):
                        nc.scalar.activation(
                            s1[:, hc, :], ph1[hc][:, 0:CAPC], AF.Relu,
                            bias=b1_sb[:, hc * NE + e: hc * NE + e + 1],
                            scale=1.0)
                    # L2: h2 rows [slot, o] (stationary = s1 slot block)
                    ph2 = phB.tile([128, NT, O], F32, tag="phB",
                                   name=f"ph2_{e}")
                    for t in range(NT):
                        w = L2W[t]
                        for hc in range(HC):
                            nc.tensor.matmul(
                                ph2[0:w, t, :],
                                s1[:, hc, t * 128:t * 128 + w],
                                w2_t[:, hc, :],
                                start=(hc == 0), stop=(hc == HC - 1),
                                skip_group_check=True)
                    for t in range(NT):
                        w = L2W[t]
                        nc.vector.tensor_scalar_mul(
                            h2g[0:w, el, t, :], ph2[0:w, t, :],
                            g_pm[0:w, e, t:t + 1])
                # scatter-accumulate per expert (static trash-token pads)
                for el in range(EPG):
                    e = g * EPG + el
                    ae, ao = acc[e % NACC]
                    nc.gpsimd.dma_scatter_add(
                        ae[:], h2g[:, el, :, :], sel16s[:, e, :],
                        num_idxs=CAP, num_idxs_reg=CAP, elem_size=O,
                        sbuf_tokens_per_rank=128, parity_reg=0,
                        out_ap_other=ao[:])

            xgp_cm.__exit__(None, None, None)

            # ---------------- merge + store ----------------
            for par in range(2):
                a0 = acc[0][par]
                for s in range(1, NACC):
                    nc.vector.tensor_add(a0[:, 0:NBT // 2, :],
                                         a0[:, 0:NBT // 2, :],
                                         acc[s][par][:, 0:NBT // 2, :])
            out_v = out_d.rearrange("(t p) o -> p t o", p=128)
            for bt in range(NBT):
                nc.sync.dma_start(out_v[:, bt, :],
                                  acc[0][bt % 2][:, bt // 2, :])

    nc.compile()
    return nc


def _prep_host(gate_b, expert_b1):
    gb = np.ascontiguousarray(np.asarray(gate_b, dtype=np.float32).reshape(NE, 1))
    b1 = np.asarray(expert_b1, dtype=np.float32)          # [64, 256]
    b1p = np.ascontiguousarray(
        b1.reshape(NE, HC, 128).transpose(2, 1, 0).reshape(128, HC * NE))
    return gb, b1p


def _consts():
    ident = np.eye(128, dtype=np.float32)
    # u129: strict upper triangular (t < c) cols 0..127, col 128 = ones
    t = np.arange(128)
    u = np.zeros((128, 130), dtype=np.float32)
    u[:, 0:128] = (t[:, None] < t[None, :]).astype(np.float32)
    u[:, 128] = 1.0
    u = u.astype(_BF16)
    # iota16: token id + 1, replicated across expert partitions
    iota16 = np.tile((np.arange(BS) + 1).astype(np.int16)[None, :], (NE, 1))
    # emat[k, q, m] = 1 iff k == 16*q + (m % 16)  (wrap-16 row picker,
    # replicated over the 8 output partition groups)
    k = np.arange(128)[:, None, None]
    q = np.arange(8)[None, :, None]
    m = np.arange(128)[None, None, :]
    emat = (k == 16 * q + (m % 16)).astype(np.float32)
    return ident, u, iota16, emat


def kernel(x, gate_w, gate_b, expert_w1, expert_b1, expert_w2, expert_b2, k):
    assert int(k) == TOPK
    assert _BF16 is not None, "ml_dtypes required for bf16 staging"
    if "nc" not in _CACHE:
        _CACHE["nc"] = _build()
    nc = _CACHE["nc"]

    x = np.asarray(x, dtype=np.float32)
    gw = np.ascontiguousarray(np.asarray(gate_w, dtype=np.float32))
    w1 = np.ascontiguousarray(
        np.asarray(expert_w1, dtype=np.float32).astype(_BF16))
    w2 = np.ascontiguousarray(
        np.asarray(expert_w2, dtype=np.float32).astype(_BF16))
    b2 = np.ascontiguousarray(np.asarray(expert_b2, dtype=np.float32))
    gb, b1p = _prep_host(gate_b, expert_b1)
    ident, u, iota16, emat = _consts()

    # capacity sanity check against the actual routing
    logits = x @ gw + np.asarray(gate_b, dtype=np.float32)
    kth = np.partition(logits, NE - TOPK, axis=1)[:, NE - TOPK]
    mask = logits >= kth[:, None]
    for c in range(NCORES):
        counts = mask[c * BS:(c + 1) * BS].sum(axis=0)
        if counts.max() > CAPC:
            raise RuntimeError(f"expert capacity {CAPC} exceeded: {counts.max()}")

    in_maps = []
    for c in range(NCORES):
        xs = x[c * BS:(c + 1) * BS]
        xt = np.ascontiguousarray(xs.T)
        xb = np.ascontiguousarray(xs.astype(_BF16))
        in_maps.append({"xt": xt, "xb": xb, "gw": gw, "gb": gb, "w1": w1,
                        "b1p": b1p, "w2": w2, "b2": b2, "idn": ident,
                        "u129": u, "iot16": iota16, "emat": emat})

    r = bass_utils.run_bass_kernel_spmd(nc, in_maps, core_ids=list(range(NCORES)))
    _CACHE["last_result"] = r
    return np.concatenate([m["out"] for m in r.results], axis=0)


# revision 46
# speedup vs baseline: 1.8080x; 1.8080x over previous
"""MoE model (64 experts, top-24 routing) on 8 Trainium2 NeuronCores.

Strategy: data-parallel shard of the 8192-token batch (1024 tokens/core).
Each core:
  - computes gate logits in exact fp32 (top-k selection fidelity),
  - top-24 masked-softmax routing weights via DVE max8/match_replace,
  - runs all 64 expert MLPs densely in float32r (TF32-like, full PE rate),
    streaming expert weights from HBM,
  - folds routing weights into the relu'd hidden activations (so layer-2
    matmuls accumulate the routing-weighted expert sum directly in PSUM
    across all 64 experts),
  - expert biases: b1 fused into the ReLU activation (per-partition bias),
    b2 applied as routing_weights @ b2 matmul opening the PSUM accumulation
    (softmax weights sum to 1 over selected experts, 0 elsewhere).

Layout notes (per core):
  xT   [1024(i), 1024(b)] : x shard transposed host-side
  L1:  h1T  [128h, 512b] = w1_chunk[128i,128h].T @ xT_chunk[128i,512b]
  s1s  = relu(h1T + b1) * wroute[b, e]   (broadcast along h)
  L2:  h2T  [128o, 512b] += w2_chunk[128h,128o].T @ s1s_chunk[128h,512b]
  out  = transpose(h2T) per 128x128 block at the end.
"""

import sys
import types

import numpy as np

import concourse.bass as bass
import concourse.tile as tile
import concourse.mybir as mybir
from concourse import bacc, bass_utils, masks

# bass_utils imports antenv.axon_hooks when BASS_TRACE=1; some images lack it.
# Provide a best-effort shim so tracing degrades instead of crashing.
try:
    import antenv.axon_hooks  # noqa: F401
except ImportError:
    try:
        import contextlib
        import ctypes

        def _make_hook():
            try:
                lib = ctypes.CDLL("/opt/axon/libaxon_pjrt.so")
            except OSError:
                return None
            if not hasattr(lib, "axon_start_nrt_profile"):
                return None
            lib.axon_start_nrt_profile.argtypes = [
                ctypes.POINTER(ctypes.c_int64), ctypes.c_size_t]
            lib.axon_start_nrt_profile.restype = ctypes.c_int64
            lib.axon_stop_nrt_profile.argtypes = [ctypes.c_char_p]
            lib.axon_stop_nrt_profile.restype = ctypes.c_int64

            @contextlib.contextmanager
            def _hook(output_dir, device_ids):
                import jax
                jax.devices()
                if device_ids:
                    ids = (ctypes.c_int64 * len(device_ids))(*device_ids)
                    rc = lib.axon_start_nrt_profile(ids, len(device_ids))
                else:
                    rc = lib.axon_start_nrt_profile(None, 0)
                if rc != 0:
                    raise RuntimeError(f"axon_start_nrt_profile rc={rc}")
                try:
                    yield
                finally:
                    lib.axon_stop_nrt_profile(str(output_dir).encode())

            return _hook

        _mod = types.ModuleType("antenv.axon_hooks")
        _mod.get_axon_ntff_profile_hook = _make_hook
        _mod.set_axon_ntff_profile_hook = lambda h: None
        sys.modules["antenv.axon_hooks"] = _mod
    except Exception:
        pass

F32 = mybir.dt.float32
F32R = mybir.dt.float32r
AF = mybir.ActivationFunctionType
ALU = mybir.AluOpType
AX = mybir.AxisListType

NCORES = 8
B = 8192
D = 1024          # input dim
H = 256           # hidden dim
O = 256           # output dim
NE = 64           # experts
TOPK = 24
BS = B // NCORES  # tokens per core (1024)
NBT = BS // 128   # b-tiles per core (8)
NG = BS // 512    # 512-token groups per core (2)
KC = D // 128     # contraction chunks for layer 1 (8)
HC = H // 128     # contraction chunks for layer 2 (2)
OC = O // 128     # output chunks (2)

_CACHE = {}


def _build():
    nc = bacc.Bacc("TRN2", target_bir_lowering=False, debug=False,
                   num_devices=NCORES)

    xt_d = nc.dram_tensor("xt", (D, BS), F32, kind="ExternalInput").ap()
    gw_d = nc.dram_tensor("gw", (D, NE), F32, kind="ExternalInput").ap()
    gb_d = nc.dram_tensor("gb", (NE, 1), F32, kind="ExternalInput").ap()
    w1_d = nc.dram_tensor("w1", (NE, D, H), F32R, kind="ExternalInput").ap()
    b1_d = nc.dram_tensor("b1p", (128, HC * NE), F32, kind="ExternalInput").ap()
    w2_d = nc.dram_tensor("w2", (NE, H, O), F32R, kind="ExternalInput").ap()
    b2_d = nc.dram_tensor("b2", (NE, O), F32, kind="ExternalInput").ap()
    out_d = nc.dram_tensor("out", (BS, O), F32, kind="ExternalOutput").ap()

    with tile.TileContext(nc) as tc:
        with tc.tile_pool(name="res", bufs=1) as res, \
             tc.tile_pool(name="w1p", bufs=3) as w1p, \
             tc.tile_pool(name="w2p", bufs=3) as w2p, \
             tc.tile_pool(name="s1p", bufs=3) as s1p, \
             tc.tile_pool(name="s1sp", bufs=3) as s1sp, \
             tc.tile_pool(name="wbp", bufs=3) as wbp, \
             tc.tile_pool(name="rt", bufs=2) as rt, \
             tc.tile_pool(name="ph1p", bufs=4, space="PSUM") as ph1p, \
             tc.tile_pool(name="ph2p", bufs=1, space="PSUM") as ph2p:

            # ---------------- resident loads ----------------
            # DMAs serialize on the Sync sequencer: emit gate inputs first and
            # chunk the x transfers so gate/L1 matmuls stream with the DMAs.
            gw_sb = res.tile([128, KC, NE], F32)
            nc.sync.dma_start(gw_sb[:], gw_d.rearrange("(c p) n -> p c n", p=128))
            gb_sb = res.tile([NE, 1], F32)
            nc.sync.dma_start(gb_sb[:], gb_d[:])

            def load_expert(e):
                w1_t = w1p.tile([128, KC, H], F32R, tag="w1", name=f"w1_{e}")
                nc.sync.dma_start(
                    w1_t[:], w1_d[e].rearrange("(c p) h -> p c h", p=128))
                w2_t = w2p.tile([128, HC, O], F32R, tag="w2", name=f"w2_{e}")
                nc.sync.dma_start(
                    w2_t[:], w2_d[e].rearrange("(c p) o -> p c o", p=128))
                return w1_t, w2_t

            # split the two x loads into interleaved half-batches so the
            # L1 moving operand lands ~7us earlier and L1(0) starts at the
            # tail of the gate matmuls instead of idling on the DMA queue
            xt_f = res.tile([128, KC, BS], F32)       # gate moving operand
            xt_r = res.tile([128, KC, BS], F32R)      # L1 moving operand
            for ic in range(KC // 2):
                nc.sync.dma_start(xt_f[:, ic, :],
                                  xt_d[ic * 128:(ic + 1) * 128, :])
            preload = {0: load_expert(0)}
            for ic in range(KC // 2):
                nc.sync.dma_start(
                    xt_r[:, ic, :],
                    xt_d.bitcast(F32R)[ic * 128:(ic + 1) * 128, :])
            for ic in range(KC // 2, KC):
                nc.sync.dma_start(xt_f[:, ic, :],
                                  xt_d[ic * 128:(ic + 1) * 128, :])
            preload[1] = load_expert(1)
            for ic in range(KC // 2, KC):
                nc.sync.dma_start(
                    xt_r[:, ic, :],
                    xt_d.bitcast(F32R)[ic * 128:(ic + 1) * 128, :])
            b1_sb = res.tile([128, HC * NE], F32)
            nc.sync.dma_start(b1_sb[:], b1_d[:])
            b2_sb = res.tile([NE, O], F32)
            nc.sync.dma_start(b2_sb[:], b2_d[:])
            ident = res.tile([128, 128], F32)
            masks.make_identity(nc, ident[:])

            g_sb = res.tile([128, NBT, NE], F32)       # gate logits
            wroute = res.tile([128, NBT, NE], F32)     # routing weights
            wrouteT = res.tile([64, NBT, 128], F32)
            accT = res.tile([128, NG * OC, 512], F32)  # h2T evacuated
            acc = res.tile([128, NBT, O], F32)         # final [b, o]

            # h2T accumulator: 4 banks resident for the whole expert loop
            ph2acc = ph2p.tile([128, NG * OC, 512], F32, tag="ph2acc")

            # ---------------- gate logits (exact fp32) ----------------
            # gw stationary (LDW hides under the 4-cyc/row fp32 matmuls),
            # xt_f moving at N=512; output gateT [64n, 512b], bias folded
            # into the per-partition ACT evacuation, then PE-transposed.
            gT_sb = res.tile([64, NG, 512], F32)
            for g in range(NG):
                pgt = ph1p.tile([128, 512], F32, tag="ph1", name=f"pgt_{g}")
                for ic in range(KC):
                    nc.tensor.matmul(
                        pgt[0:NE, :],
                        gw_sb[:, ic, :],
                        xt_f[:, ic, g * 512:(g + 1) * 512],
                        start=(ic == 0), stop=(ic == KC - 1))
                nc.scalar.activation(gT_sb[:, g, :], pgt[0:NE, :],
                                     AF.Identity, bias=gb_sb[:], scale=1.0)
                # transpose this group's b-tiles immediately so the DVE
                # routing chain starts before the other group's gate matmuls
                for btl in range(4):
                    bt = g * 4 + btl
                    ptg = ph1p.tile([128, 512], F32, tag="ph1",
                                    name=f"ptg_{bt}")
                    nc.tensor.transpose(
                        ptg[:, 0:NE],
                        gT_sb[:, g, btl * 128:(btl + 1) * 128],
                        ident[0:NE, 0:NE])
                    nc.scalar.copy(g_sb[:, bt, :], ptg[:, 0:NE])

            # ---------------- top-24 masked softmax ----------------
            for bt in range(NBT):
                g = g_sb[:, bt, :]
                m8 = rt.tile([128, 3, 8], F32, tag="m8")
                gwk = rt.tile([128, 3, NE], F32, tag="gwk")
                nc.vector.max(m8[:, 0, :], g)
                nc.vector.match_replace(gwk[:, 0, :], m8[:, 0, :], g, -1e30)
                nc.vector.max(m8[:, 1, :], gwk[:, 0, :])
                nc.vector.match_replace(gwk[:, 1, :], m8[:, 1, :], gwk[:, 0, :], -1e30)
                nc.vector.max(m8[:, 2, :], gwk[:, 1, :])
                nc.vector.match_replace(gwk[:, 2, :], m8[:, 2, :], gwk[:, 1, :], -1e30)
                maskt = rt.tile([128, NE], F32, tag="maskt")
                nc.vector.tensor_scalar(maskt[:], gwk[:, 2, :], -1e29, None,
                                        op0=ALU.is_lt)
                negm1 = rt.tile([128, 1], F32, tag="negm1")
                nc.vector.tensor_scalar_mul(negm1[:], m8[:, 0, 0:1], -1.0)
                e_sb = rt.tile([128, NE], F32, tag="e_sb")
                nc.scalar.activation(e_sb[:], g, AF.Exp, bias=negm1[:], scale=1.0)
                em = rt.tile([128, NE], F32, tag="em")
                nc.vector.tensor_mul(em[:], e_sb[:], maskt[:])
                ssum = rt.tile([128, 1], F32, tag="ssum")
                nc.vector.reduce_sum(ssum[:], em[:], axis=AX.X)
                rsum = rt.tile([128, 1], F32, tag="rsum")
                nc.vector.reciprocal(rsum[:], ssum[:])
                nc.vector.tensor_scalar_mul(wroute[:, bt, :], em[:], rsum[:])

            # wrouteT transposes + b2 bias matmuls: emitted after L1+relu of
            # expert 0 so the PE covers the routing chain's tail.
            def emit_route_t_and_bias():
                for bt in range(NBT):
                    ptr_ = ph1p.tile([128, 512], F32, tag="ph1",
                                     name=f"ptr_{bt}")
                    nc.tensor.transpose(ptr_[0:64, 0:128], wroute[:, bt, :],
                                        ident[:])
                    nc.scalar.copy(wrouteT[:, bt, :], ptr_[0:64, 0:128])
                for g in range(NG):
                    for oc in range(OC):
                        nc.tensor.matmul(
                            ph2acc[:, g * OC + oc, :],
                            b2_sb[:, oc * 128:(oc + 1) * 128],
                            wrouteT[:, g * 4:(g + 1) * 4, :],
                            start=True, stop=False, skip_group_check=True)

            # ---------------- dense expert loop (software-pipelined) ------
            def emit_l1(e, w1_t):
                # g innermost: one stationary load (w1 chunk) feeds both
                # 512-token groups -> half the LDWEIGHTS traffic
                ph1 = [[ph1p.tile([128, 512], F32, tag="ph1",
                                  name=f"ph1_{e}_{g}_{i}")
                        for i in range(HC)] for g in range(NG)]
                for hc in range(HC):
                    for ic in range(KC):
                        for g in range(NG):
                            nc.tensor.matmul(
                                ph1[g][hc][:],
                                w1_t[:, ic, hc * 128:(hc + 1) * 128],
                                xt_r[:, ic, g * 512:(g + 1) * 512],
                                start=(ic == 0), stop=(ic == KC - 1))
                return ph1

            def emit_relu(e, ph1):
                s1 = []
                for g in range(NG):
                    s1_g = s1p.tile([128, HC, 512], F32, tag="s1",
                                    name=f"s1_{e}_{g}")
                    s1.append(s1_g)
                    for hc in range(HC):
                        nc.scalar.activation(
                            s1_g[:, hc, :], ph1[g][hc][:], AF.Relu,
                            bias=b1_sb[:, hc * NE + e: hc * NE + e + 1],
                            scale=1.0)
                return s1

            def emit_scale(e, s1):
                s1s = []
                for g in range(NG):
                    wb0 = wbp.tile([1, 512], F32, tag="wb0", name=f"wb0_{e}_{g}")
                    nc.sync.dma_start(wb0[:], wrouteT[e:e + 1, g * 4:(g + 1) * 4, :])
                    wb = wbp.tile([128, 512], F32, tag="wb", name=f"wb_{e}_{g}")
                    nc.gpsimd.partition_broadcast(wb[:], wb0[:])
                    s1s_g = s1sp.tile([128, HC, 512], F32R, tag="s1s",
                                      name=f"s1s_{e}_{g}")
                    s1s.append(s1s_g)
                    for hc in range(HC):
                        nc.vector.tensor_tensor(
                            s1s_g[:, hc, :], s1[g][:, hc, :], wb[:],
                            op=ALU.mult)
                return s1s

            def emit_l2(e, w2_t, s1s, last):
                for hc in range(HC):
                    for oc in range(OC):
                        for g in range(NG):
                            nc.tensor.matmul(
                                ph2acc[:, g * OC + oc, :],
                                w2_t[:, hc, oc * 128:(oc + 1) * 128],
                                s1s[g][:, hc, :],
                                start=False,
                                stop=(last and hc == HC - 1),
                                skip_group_check=True)

            w1_t0, w2_t0 = preload[0]
            ph1_0 = emit_l1(0, w1_t0)
            s1_0 = emit_relu(0, ph1_0)
            emit_route_t_and_bias()
            prev = (0, w2_t0, emit_scale(0, s1_0))
            for e in range(1, NE):
                w1_t, w2_t = preload[e] if e in preload else load_expert(e)
                ph1 = emit_l1(e, w1_t)
                s1s = emit_scale(e, emit_relu(e, ph1))
                emit_l2(prev[0], prev[1], prev[2], last=False)
                prev = (e, w2_t, s1s)
            emit_l2(prev[0], prev[1], prev[2], last=True)

            # ---------------- evacuate + transpose back + store ----------
            out_v = out_d.rearrange("(t p) o -> p t o", p=128)
            for g in range(NG):
                for oc in range(OC):
                    j = g * OC + oc
                    nc.vector.tensor_copy(accT[:, j, :], ph2acc[:, j, :])
                    for btl in range(4):
                        bt = g * 4 + btl
                        ptt = ph1p.tile([128, 512], F32, tag="ph1",
                                        name=f"ptt_{g}_{oc}_{btl}")
                        nc.tensor.transpose(
                            ptt[:, 0:128],
                            accT[:, j, btl * 128:(btl + 1) * 128],
                            ident[:])
                        nc.scalar.copy(acc[:, bt, oc * 128:(oc + 1) * 128],
                                       ptt[:, 0:128])
                    nc.sync.dma_start(
                        out_v[:, g * 4:(g + 1) * 4, oc * 128:(oc + 1) * 128],
                        acc[:, g * 4:(g + 1) * 4, oc * 128:(oc + 1) * 128])

    nc.compile()
    return nc


def _prep_host(gate_b, expert_b1):
    gb = np.ascontiguousarray(np.asarray(gate_b, dtype=np.float32).reshape(NE, 1))
    b1 = np.asarray(expert_b1, dtype=np.float32)          # [64, 256]
    b1p = np.ascontiguousarray(
        b1.reshape(NE, HC, 128).transpose(2, 1, 0).reshape(128, HC * NE))
    return gb, b1p


def kernel(x, gate_w, gate_b, expert_w1, expert_b1, expert_w2, expert_b2, k):
    assert int(k) == TOPK
    if "nc" not in _CACHE:
        _CACHE["nc"] = _build()
    nc = _CACHE["nc"]

    x = np.asarray(x, dtype=np.float32)
    gw = np.ascontiguousarray(np.asarray(gate_w, dtype=np.float32))
    w1 = np.ascontiguousarray(np.asarray(expert_w1, dtype=np.float32))
    w2 = np.ascontiguousarray(np.asarray(expert_w2, dtype=np.float32))
    b2 = np.ascontiguousarray(np.asarray(expert_b2, dtype=np.float32))
    gb, b1p = _prep_host(gate_b, expert_b1)

    in_maps = []
    for c in range(NCORES):
        xt = np.ascontiguousarray(x[c * BS:(c + 1) * BS].T)
        in_maps.append({"xt": xt, "gw": gw, "gb": gb, "w1": w1, "b1p": b1p,
                        "w2": w2, "b2": b2})

    r = bass_utils.run_bass_kernel_spmd(nc, in_maps, core_ids=list(range(NCORES)))
    _CACHE["last_result"] = r
    return np.concatenate([m["out"] for m in r.results], axis=0)

